# revision 29
# baseline (speedup 1.0000x reference)
"""CTC loss (Keras ctc_batch_cost semantics) on 8 Trainium2 NeuronCores.

v8: blank-normalized CTC DP with parity-split dense state.

Each core handles 64 sequences; 128 DP rows = 64 fwd + 64 bwd (state-reversed)
chains meeting in the middle.  Dividing every emission by the blank emission
D_t = y_blank+eps makes the blank-state multiplier exactly 1, so blank states
need NO per-step multiply; the 1/D scale rides the scalar slot of the odd
update (per-step [128,1] AP from a per-chunk reciprocal of the transposed
blank slots), and sum_t ln(1/D) is added back in the W epilogue term.  The DP
state is parity-split into dense tiles Qo[128,80] (label states), Qe[128,81]
(blank states), Rh[128,80] (skip-premultiplied odd states), giving 5 dense
in-place DVE ops per step (no strided writes, no ping-pong):

    xo  = Qo + tau*Qe           xo[1:] += Rh[:-1]
    Qe[1:] += tau*Qo            Qo = (xo*r_t) * G1_t      Rh = Qo * mn2

Producers are strictly stage-per-engine with no per-unit cross-engine side
chains (the v6 killer: DMA triggers interleaved between casts that waited on
gathers): sync issues all DMAs (loads / transpose legs, with pipeline lags),
act does plain +eps casts with the Z accumulation, gpsimd does the 81-wide
[blank, labels] gathers.  Transpose staging uses 64 per-unit DRAM tensors
(no shared-tensor WAW serialization).

  loss[b] = sum_t (ln Z + ln r) - sum_k ln z_f - sum_k ln z_b - ln P
            + 160 ln tau
"""

import functools
import os
import sys

import numpy as np

B, T, C, L = 512, 512, 128, 80
S = 2 * L + 1  # 161
BLANK = C - 1
EPS = 1e-7
NCORES = 8
BPC = B // NCORES  # 64 sequences per core
TC = 128  # time-chunk
NMC = 2  # chunks per direction (255 chain steps)
TSUB = 8  # times per partition-cell
NW = TC // TSUB  # 16 w-slots
NOCT = BPC // 8  # 8 seq-octets per core
NUNITS = NMC * 2 * NOCT  # 64 producer units (mc, dir, octet)
RN = 8
NRENORM = 31
SPAD = S + 2  # 163
SG = S + 3  # 164
SW = 81  # gather slot width: [blank D, 80 labels]
GW = TSUB * SW  # 648
IDX3W = 42  # idx cols (648/16=40.5 -> 41, padded even)
GWPAD = 672  # gather out width (num_valid must be 16-aligned)
RIDXW = 12
TAU = 0.3


def _emit_kernel(ctx, tc, ypred, idxt, maskt, mn2t, losst):
    import concourse.bass as bass  # noqa: F401
    import concourse.mybir as mybir
    from concourse.ap import AP

    nc = tc.nc
    f32 = mybir.dt.float32
    bf16 = mybir.dt.bfloat16
    Alu = mybir.AluOpType
    Act = mybir.ActivationFunctionType

    singles = ctx.enter_context(tc.tile_pool(name="singles", bufs=1))
    ypool = ctx.enter_context(tc.tile_pool(name="ypool", bufs=8))
    bpool = ctx.enter_context(tc.tile_pool(name="bpool", bufs=4))
    gpool = ctx.enter_context(tc.tile_pool(name="gpool", bufs=8))
    small = ctx.enter_context(tc.tile_pool(name="small", bufs=2))
    finp = ctx.enter_context(tc.tile_pool(name="finp", bufs=8))
    epi = ctx.enter_context(tc.tile_pool(name="epi", bufs=1))
    psump = ctx.enter_context(tc.tile_pool(name="psum", bufs=2, space="PSUM"))

    # --- constants loaded once -------------------------------------------
    NIDX = 2 * NOCT * IDX3W + RIDXW
    idx_sb = singles.tile([128, NIDX], mybir.dt.uint16)
    nc.sync.dma_start(out=idx_sb[:, :], in_=idxt)
    m_sb = singles.tile([128, S], bf16)
    nc.sync.dma_start(out=m_sb[:, :], in_=maskt)
    mn2_sb = singles.tile([128, SW - 1], bf16)
    nc.sync.dma_start(out=mn2_sb[:, :], in_=mn2t)
    gsel = singles.tile([128, 8], f32)
    nc.sync.dma_start(out=gsel[:, :], in_=tc.gselt)
    idx_scr = singles.tile([16, 1], mybir.dt.uint16)
    nc.gpsimd.tensor_copy(out=idx_scr[:, :], in_=idx_sb[0:16, 0:1])

    # Z accumulator: col = unit*TSUB + u, unit = (mc*2+dir)*NOCT + o
    zbig = singles.tile([128, NUNITS * TSUB], f32)
    zstash = singles.tile([128, NRENORM], f32)
    # 1/D per (row, step i): filled per chunk from the transposed blank slots
    rblock = singles.tile([128, NMC * TC], f32)

    gtiles = []
    for mc in range(NMC):
        gtile = singles.tile([128, TC * SW], bf16, tag=f"gtile{mc}")
        gtiles.append(gtile)

    # --- producer pipeline -----------------------------------------------
    # unit k = (mc, d, o).  HWDGE DMAs execute in FIFO order per issuing
    # engine ring, and a consumer waiting on DMA #n waits for every DMA
    # before it in that ring — so the three DMA streams are spread across
    # all three rings: octet-PAIR loads fwd on sync / bwd on act, per-unit
    # leg1 on gpsimd's SWDGE (issued right after its gather, never blocks),
    # and one leg2 per (mc,d) group on sync.
    def pair_load(kp):
        k = 2 * kp
        mc, r = divmod(k, 2 * NOCT)
        d, o = divmod(r, NOCT)
        yb = ypool.tile([128, 2 * TSUB * C], f32, tag="yb")
        if d == 0:
            off = 8 * o * T * C + (mc * TC) * C
            wstep = TSUB * C
        else:
            off = 8 * o * T * C + ((NMC * 2 - 1 - mc) * TC) * C + (NW - 1) * TSUB * C
            wstep = -TSUB * C
        eng = nc.sync if d == 0 else nc.scalar
        for j in range(2):
            src = AP(
                ypred.tensor,
                ypred.offset + off + j * 8 * T * C,
                [[T * C, 8], [wstep, NW], [1, TSUB * C]],
            )
            eng.dma_start(
                out=yb[:, j * TSUB * C : (j + 1) * TSUB * C], in_=src
            )
        return yb

    NPAIR = NUNITS // 2
    LOOKAHEAD = 7  # pairs
    ybigs = {}
    for kp in range(LOOKAHEAD):
        ybigs[kp] = pair_load(kp)
    for k in range(NUNITS):
        kp, j = divmod(k, 2)
        if j == 0 and kp + LOOKAHEAD < NPAIR:
            ybigs[kp + LOOKAHEAD] = pair_load(kp + LOOKAHEAD)
        yb = ybigs[kp] if j == 0 else ybigs.pop(kp)
        jo = j * TSUB * C
        mc, r = divmod(k, 2 * NOCT)
        d, o = divmod(r, NOCT)
        # label gather straight off the fp32 load (no cast in this loop);
        # the eps bias is applied by the chain-side emission math being
        # tolerant: we add eps in the Z accum below and in the gathered
        # values via the casting leg1?  No: eps rides the Z accum and the
        # chain's G1; gathered values are y (eps added numerically via the
        # +eps Z path cancels).  See module docstring.
        g2 = gpool.tile([128, GWPAD], f32, tag="g2")
        nc.gpsimd.tensor_copy(out=g2[0:16, 0:1], in_=yb[0:16, jo : jo + 1])
        ib = (d * NOCT + o) * IDX3W
        nc.gpsimd.indirect_copy(
            g2[:, :], yb[:, jo : jo + TSUB * C], idx_sb[:, ib : ib + IDX3W], True
        )
        # leg1 from gp's own SWDGE ring, right after the gather it reads;
        # the DMA itself casts fp32 -> bf16 (SWDGE-only feature)
        out_v = tc.gst[2 * mc + d][8 * o : 8 * o + 8, :].rearrange(
            "g (w q) -> g w q", w=NW
        )
        nc.gpsimd.dma_start(out=out_v, in_=g2[:, 0:GW])
        # Z accumulation: free-running act side branch, consumed by nobody
        # until the epilogue (dummy bf16 output tile)
        ydum = bpool.tile([128, TSUB * C], bf16, tag="yf")
        for u in range(TSUB):
            col = k * TSUB + u
            nc.scalar.activation(
                out=ydum[:, u * C : (u + 1) * C],
                in_=yb[:, jo + u * C : jo + (u + 1) * C],
                func=Act.Copy,
                bias=EPS,
                accum_out=zbig[:, col : col + 1],
            )
        # leg2 per (mc,d) group, 4 units after the group's last gather
        if k >= 11 and (k - 11) % 8 == 0:
            _leg2(nc, tc, gtiles, (k - 11) // 8)
    _leg2(nc, tc, gtiles, 2 * NMC - 1)

    # --- the DP chain (vector only) ---------------------------------------
    NLAB = SW - 1  # 80 label states
    Qo = singles.tile([128, NLAB], bf16)
    Qe = singles.tile([128, NLAB + 1], bf16)
    xo = singles.tile([128, NLAB], bf16)
    Rh = singles.tile([128, NLAB], bf16)

    def chunk_preamble(mc):
        # rblock[:, mc*TC : (mc+1)*TC] = 1 / (blank-slots of gtile[mc] + eps)
        blf = small.tile([128, TC], f32, tag="blf")
        nc.vector.tensor_scalar(
            out=blf[:, :], in0=gtiles[mc][:, 0 : TC * SW : SW], scalar1=EPS,
            scalar2=0.0, op0=Alu.add, op1=Alu.add,
        )
        nc.vector.reciprocal(
            out=rblock[:, mc * TC : (mc + 1) * TC], in_=blf[:, :]
        )

    chunk_preamble(0)
    nc.vector.memset(Qe[:, :], 0.0)
    nc.vector.memset(Qe[:, 0:1], 1.0)
    nc.vector.memset(Qo[:, :], 0.0)
    nc.vector.tensor_scalar(
        out=Qo[:, 0:1], in0=gtiles[0][:, 1:2], scalar1=rblock[:, 0:1],
        scalar2=TAU, op0=Alu.mult, op1=Alu.mult,
    )
    nc.vector.memset(Rh[:, :], 0.0)
    nc.vector.tensor_tensor(
        out=Rh[:, 0:1], in0=Qo[:, 0:1], in1=mn2_sb[:, 0:1], op=Alu.mult
    )

    rz = None
    for i in range(1, 256):
        mc, toff = divmod(i, TC)
        if toff == 0 and mc > 0:
            chunk_preamble(mc)
        g1 = gtiles[mc][:, toff * SW + 1 : (toff + 1) * SW]
        rt = rblock[:, i : i + 1]
        measure = (i % RN == RN - 1) and i < 255
        fold = (i % RN == 0) and i >= RN
        # (a) xo = Qo + tau*Qe
        nc.vector.scalar_tensor_tensor(
            out=xo[:, :], in0=Qe[:, 0:NLAB], scalar=TAU, in1=Qo[:, :],
            op0=Alu.mult, op1=Alu.add,
        )
        # (b) xo[1:] += Rh[:-1]   (skip transitions)
        nc.vector.tensor_tensor(
            out=xo[:, 1:NLAB], in0=xo[:, 1:NLAB], in1=Rh[:, 0 : NLAB - 1],
            op=Alu.add,
        )
        # (e) Qe[1:] += tau*Qo  (in place; Qe[0] is constant between folds)
        if measure:
            ze = small.tile([128, 1], f32, tag="ze")
            nc.vector.scalar_tensor_tensor(
                out=Qe[:, 1 : NLAB + 1], in0=Qo[:, :], scalar=TAU,
                in1=Qe[:, 1 : NLAB + 1], op0=Alu.mult, op1=Alu.add,
                accum_out=ze[:, :],
            )
        else:
            nc.vector.scalar_tensor_tensor(
                out=Qe[:, 1 : NLAB + 1], in0=Qo[:, :], scalar=TAU,
                in1=Qe[:, 1 : NLAB + 1], op0=Alu.mult, op1=Alu.add,
            )
        # (c) Qo = (xo * r_t) * G1  -- 1/D rides the scalar slot; renorm
        # folds multiply rz in as well
        if fold:
            rzr = small.tile([128, 1], f32, tag="rzr")
            nc.vector.tensor_tensor(
                out=rzr[:, :], in0=rz[:, :], in1=rt, op=Alu.mult
            )
            nc.vector.scalar_tensor_tensor(
                out=Qo[:, :], in0=xo[:, :], scalar=rzr[:, :], in1=g1,
                op0=Alu.mult, op1=Alu.mult,
            )
            # (f) fold even states too
            nc.vector.tensor_scalar(
                out=Qe[:, :], in0=Qe[:, :], scalar1=rz[:, :], scalar2=1.0,
                op0=Alu.mult, op1=Alu.mult,
            )
        elif measure:
            zo = small.tile([128, 1], f32, tag="zo")
            nc.vector.scalar_tensor_tensor(
                out=Qo[:, :], in0=xo[:, :], scalar=rt, in1=g1,
                op0=Alu.mult, op1=Alu.mult, accum_out=zo[:, :],
            )
        else:
            nc.vector.scalar_tensor_tensor(
                out=Qo[:, :], in0=xo[:, :], scalar=rt, in1=g1,
                op0=Alu.mult, op1=Alu.mult,
            )
        # (d) Rh = Qo * mn2
        nc.vector.tensor_tensor(
            out=Rh[:, :], in0=Qo[:, :], in1=mn2_sb[:, 0:NLAB], op=Alu.mult
        )
        if measure:
            kk = i // RN
            nc.vector.tensor_tensor(
                out=zstash[:, kk : kk + 1], in0=zo[:, :], in1=ze[:, :], op=Alu.add
            )
            rz = small.tile([128, 1], f32, tag="rz")
            nc.vector.reciprocal(out=rz[:, :], in_=zstash[:, kk : kk + 1])

    # --- epilogue ---------------------------------------------------------
    # rebuild interleaved full state (state s at col s+2, as v6)
    Qfull = singles.tile([128, SPAD], bf16)
    nc.vector.memset(Qfull[:, :], 0.0)
    nc.vector.tensor_copy(out=Qfull[:, 2:SPAD:2], in_=Qe[:, :])
    nc.vector.tensor_copy(out=Qfull[:, 3:SPAD:2], in_=Qo[:, :])
    # beta' step on bwd rows: bt = g~ + tau*g~[-1] + m~t*g~[-2]
    bx = epi.tile([64, S], bf16, tag="bx")
    nc.vector.scalar_tensor_tensor(
        out=bx[:, :], in0=Qfull[64:128, 1 : S + 1], scalar=TAU,
        in1=Qfull[64:128, 2:SPAD], op0=Alu.mult, op1=Alu.add,
    )
    by = epi.tile([64, S], bf16, tag="by")
    nc.vector.tensor_tensor(
        out=by[:, :], in0=m_sb[64:128, :], in1=Qfull[64:128, 0:S], op=Alu.mult
    )
    btfull = singles.tile([128, SG], bf16)
    nc.vector.memset(btfull[:, :], 0.0)
    nc.vector.tensor_tensor(
        out=btfull[64:128, 0:S], in0=bx[:, :], in1=by[:, :], op=Alu.add
    )
    # state-reversal gather
    btrev = singles.tile([128, SG], bf16)
    nc.gpsimd.tensor_copy(out=btrev[0:16, 0:1], in_=btfull[0:16, 0:1])
    jr = 2 * NOCT * IDX3W
    nc.gpsimd.indirect_copy(
        btrev[:, :], btfull[:, :], idx_sb[:, jr : jr + RIDXW], True
    )
    balign = epi.tile([64, S], bf16, tag="balign")
    nc.sync.dma_start(out=balign[:, :], in_=btrev[64:128, 0:S])
    # join dot: Phat[b] = sum_s alpha[b,s] * balign[b,s]
    pjunk = epi.tile([64, S], bf16, tag="pjunk")
    phat = finp.tile([64, 1], f32, tag="fin")
    nc.vector.scalar_tensor_tensor(
        out=pjunk[:, :], in0=Qfull[0:64, 2:SPAD], scalar=1.0, in1=balign[:, :],
        op0=Alu.mult, op1=Alu.mult, accum_out=phat[:, :],
    )
    lnp = finp.tile([64, 1], f32, tag="fin")
    nc.scalar.activation(out=lnp[:, :], in_=phat[:, :], func=Act.Ln)

    # renorm scale logs
    lnzt = epi.tile([128, NRENORM], f32, tag="lnzt")
    nc.scalar.activation(out=lnzt[:, :], in_=zstash[:, :], func=Act.Ln)
    rfull = epi.tile([128, 1], f32, tag="rfull")
    nc.vector.reduce_sum(out=rfull[:, :], in_=lnzt[:, :], axis=mybir.AxisListType.X)
    rb = finp.tile([64, 1], f32, tag="fin")
    nc.sync.dma_start(out=rb[:, :], in_=rfull[64:128, :])
    rsum = finp.tile([64, 1], f32, tag="fin")
    nc.vector.tensor_tensor(
        out=rsum[:, :], in0=rfull[0:64, :], in1=rb[:, :], op=Alu.add
    )

    # softmax normalizer W: ln(Z~), reduce (u) then (mcd), PE group-sum over w
    NMCD = 2 * NMC
    lnZ = singles.tile([128, NMCD * NOCT * TSUB], f32)
    nc.scalar.activation(out=lnZ[:, :], in_=zbig[:, :], func=Act.Ln)
    wt1 = singles.tile([128, NMCD * NOCT], f32)
    lv = lnZ[:, :].rearrange("p (m o u) -> p (m o) u", m=NMCD, o=NOCT)
    nc.vector.reduce_sum(out=wt1[:, :], in_=lv, axis=mybir.AxisListType.X)
    wsum3 = singles.tile([128, NOCT], f32)
    lv2 = wt1[:, :].rearrange("p (m o) -> p o m", m=NMCD)
    nc.vector.reduce_sum(out=wsum3[:, :], in_=lv2, axis=mybir.AxisListType.X)
    psw = psump.tile([8, 8], f32, tag="ps1")
    nc.tensor.matmul(psw[:, :], lhsT=gsel[:, :], rhs=wsum3[:, :], start=True, stop=True)
    wsb = epi.tile([8, 8], f32, tag="wsb")
    nc.vector.tensor_copy(out=wsb[:, :], in_=psw[:, :])
    wb = finp.tile([BPC, 1], f32, tag="fin")
    for o in range(NOCT):
        nc.sync.dma_start(out=wb[8 * o : 8 * o + 8, :], in_=wsb[:, o : o + 1])

    # W_r = sum_t ln(1/D_t) per chain row, fwd+bwd folded per seq
    lnrt = epi.tile([128, NMC * TC], f32, tag="lnrt")
    nc.scalar.activation(out=lnrt[:, :], in_=rblock[:, :], func=Act.Ln)
    wrfull = epi.tile([128, 1], f32, tag="wrfull")
    nc.vector.reduce_sum(out=wrfull[:, :], in_=lnrt[:, :], axis=mybir.AxisListType.X)
    wrb = finp.tile([64, 1], f32, tag="fin")
    nc.sync.dma_start(out=wrb[:, :], in_=wrfull[64:128, :])
    wradd = finp.tile([64, 1], f32, tag="fin")
    nc.vector.tensor_tensor(
        out=wradd[:, :], in0=wrfull[0:64, :], in1=wrb[:, :], op=Alu.add
    )

    # loss = W_Z + W_r - Rsum - lnP + 160 ln tau
    t0 = finp.tile([BPC, 1], f32, tag="fin")
    nc.vector.tensor_tensor(out=t0[:, :], in0=wb[:, :], in1=wradd[:, :], op=Alu.add)
    t1 = finp.tile([BPC, 1], f32, tag="fin")
    nc.vector.tensor_tensor(out=t1[:, :], in0=t0[:, :], in1=rsum[:, :], op=Alu.subtract)
    t2 = finp.tile([BPC, 1], f32, tag="fin")
    nc.vector.tensor_tensor(out=t2[:, :], in0=t1[:, :], in1=lnp[:, :], op=Alu.subtract)
    lt = finp.tile([BPC, 1], f32, tag="fin")
    nc.vector.tensor_scalar(
        out=lt[:, :], in0=t2[:, :], scalar1=float(S - 1) * float(np.log(TAU)),
        scalar2=0.0, op0=Alu.add, op1=Alu.add,
    )
    nc.sync.dma_start(out=losst, in_=lt[:, :])


def _leg2(nc, tc, gtiles, gidx):
    # gst[(mc,d)] [64 rows, TC*SW] -> gtile[mc] rows [64d : 64d+64]
    mc, d = divmod(gidx, 2)
    nc.sync.dma_start(
        out=gtiles[mc][64 * d : 64 * d + 64, :], in_=tc.gst[gidx][:, :]
    )


@functools.lru_cache(maxsize=4)
def _build():
    from contextlib import ExitStack

    import concourse.bacc as bacc
    import concourse.mybir as mybir
    import concourse.tile as tile

    nc = bacc.Bacc(trn_type="TRN2", target_bir_lowering=False)
    ypred = nc.dram_tensor("y_pred", [BPC, T, C], mybir.dt.float32, kind="ExternalInput")
    NIDX = 2 * NOCT * IDX3W + RIDXW
    idxt = nc.dram_tensor("idx", [128, NIDX], mybir.dt.uint16, kind="ExternalInput")
    maskt = nc.dram_tensor("mask", [128, S], mybir.dt.bfloat16, kind="ExternalInput")
    mn2t = nc.dram_tensor("mn2", [128, SW - 1], mybir.dt.bfloat16, kind="ExternalInput")
    gselt = nc.dram_tensor("gsel", [128, 8], mybir.dt.float32, kind="ExternalInput")
    losst = nc.dram_tensor("loss", [BPC, 1], mybir.dt.float32, kind="ExternalOutput")
    gst = [
        nc.dram_tensor(f"gst{g}", [64, TC * SW], mybir.dt.bfloat16, kind="Internal")
        for g in range(2 * NMC)
    ]
    with tile.TileContext(nc) as tc:
        tc.gselt = gselt[:, :]
        tc.gst = gst
        with ExitStack() as ctx:
            _emit_kernel(
                ctx, tc, ypred[:, :, :], idxt[:, :], maskt[:, :], mn2t[:, :],
                losst[:, :],
            )
    nc.compile()
    return nc


def _host_prep(y_true):
    """Dense 80-slot octet gather indices (fwd, bwd time+state reversed,
    epilogue state reversal), tau^2-scaled masks, and the dense skip mask."""
    import ml_dtypes

    bf = ml_dtypes.bfloat16
    lab = np.asarray(y_true).astype(np.int64)
    ext_m = np.zeros((B, S), dtype=np.float32)
    ext_m[:, 1] = 1.0
    ext_m[:, 3::2] = (lab[:, 1:] != lab[:, :-1]).astype(np.float32)
    mt = np.zeros((B, S), dtype=np.float32)  # m~[s'] = m[162-s']
    sp = np.arange(2, S)
    mt[:, sp] = ext_m[:, 162 - sp]
    t2 = np.float32(TAU * TAU)

    NIDX = 2 * NOCT * IDX3W + RIDXW
    p = np.arange(128)
    idx_all, mask_all, mn2_all = [], [], []
    for kcore in range(NCORES):
        base = kcore * BPC
        idx = np.zeros((128, NIDX), dtype=np.uint16)
        for d in range(2):
            for o in range(NOCT):
                ib = (d * NOCT + o) * IDX3W
                for f in range(IDX3W):
                    pos = f * 16 + (p % 16)  # position in the 324-list
                    valid = pos < GW
                    u = np.minimum(pos // SW, TSUB - 1)
                    slot = pos % SW
                    g = p // 16
                    seq = base + 8 * o + g
                    ueff = u if d == 0 else (TSUB - 1 - u)
                    val = np.zeros(128, dtype=np.uint16)
                    bl = valid & (slot == 0)
                    lb = valid & (slot >= 1)
                    val[bl] = (ueff[bl] * C + BLANK).astype(np.uint16)
                    if d == 0:
                        val[lb] = (
                            ueff[lb] * C + lab[seq[lb], slot[lb] - 1]
                        ).astype(np.uint16)
                    else:
                        val[lb] = (
                            ueff[lb] * C + lab[seq[lb], L - slot[lb]]
                        ).astype(np.uint16)
                    idx[:, ib + f] = val
        jr = 2 * NOCT * IDX3W
        for f in range(RIDXW):
            pos = f * 16 + (p % 16)
            valid = pos < S
            col = np.zeros(128, dtype=np.uint16)
            col[valid] = (S - 1) - pos[valid]
            idx[:, jr + f] = col
        idx_all.append(idx)

        mask = np.zeros((128, S), dtype=np.float32)
        mask[0:64] = ext_m[base : base + BPC] * t2
        mask[64:128] = mt[base : base + BPC] * t2
        mask_all.append(mask.astype(bf))
        # dense skip mask for Rh: mn2[r, j] = mask[r, 2j+3], 0 at j=79
        modd = mask[:, 1::2]
        mn2 = np.concatenate([modd[:, 1:80], np.zeros((128, 1), np.float32)], axis=1)
        mn2_all.append(np.ascontiguousarray(mn2).astype(bf))
    return idx_all, mask_all, mn2_all


def gsel_host():
    g = np.zeros((128, 8), dtype=np.float32)
    for gg in range(8):
        g[16 * gg : 16 * gg + 16, gg] = 1.0
    return g


def kernel(y_true, y_pred):
    from concourse.bass_utils import run_bass_kernel_spmd

    y_pred = np.ascontiguousarray(np.asarray(y_pred), dtype=np.float32)
    idx_all, mask_all, mn2_all = _host_prep(y_true)

    nc = _build()
    in_maps = []
    for k in range(NCORES):
        b0 = k * BPC
        in_maps.append(
            {
                "y_pred": np.ascontiguousarray(y_pred[b0 : b0 + BPC]),
                "idx": idx_all[k],
                "mask": mask_all[k],
                "mn2": mn2_all[k],
                "gsel": gsel_host(),
            }
        )
    res = run_bass_kernel_spmd(
        nc,
        in_maps,
        core_ids=list(range(NCORES)),
        trace=bool(int(os.environ.get("CTC_TRACE", "0"))),
    )
    out = np.concatenate([r["loss"] for r in res.results], axis=0)
    if res.exec_time_ns is not None:
        print(f"HW exec time: {res.exec_time_ns} ns", file=sys.stderr)
    return out.astype(np.float32)


# revision 30
# speedup vs baseline: 1.0688x; 1.0688x over previous
"""CTC loss (Keras ctc_batch_cost semantics) on 8 Trainium2 NeuronCores.

v10: blank-normalized CTC DP, parity-split dense state, cast-free producers.

Each core handles 64 sequences; 128 DP rows = 64 fwd + 64 bwd (state-reversed)
chains meeting in the middle.  Dividing every emission by the blank emission
D_t makes the blank-state multiplier exactly 1, so blank states need NO
per-step multiply; the 1/D scale rides the scalar slot of the odd update
(per-step [128,1] AP from a per-chunk reciprocal of the transposed blank
slots), and sum_t ln(1/D) is added back via the W_r epilogue term (any
consistent positive scale cancels exactly between the path product and that
term).  The DP state is parity-split into dense tiles Qo[128,80] (label
states), Qe[128,81] (blank states), Rh[128,80] (skip-premultiplied odd
states), giving 5 dense in-place DVE ops per step (no strided writes, no
ping-pong buffers):

    xo  = Qo + tau*Qe           xo[1:] += Rh[:-1]
    Qe[1:] += tau*Qo            Qo = (xo*r_t) * G1_t      Rh = Qo * mn2

Producer pipeline per unit (mc, dir, octet): the 81-wide [blank, labels]
gathers (gpsimd indirect_copy) read the fp32 load tiles DIRECTLY — no cast
sits between load and gather — and the transpose leg1 DMA does the
fp32->bf16 cast itself (a SWDGE-only DMA feature).  The Z accumulation runs
as a free-running act-engine side branch into a dummy tile nobody consumes.
HWDGE DMAs execute in FIFO order per issuing-engine ring, so the DMA streams
are spread across all three rings: octet-pair loads fwd on sync / bwd on
act, leg1 on gpsimd's SWDGE right after each gather, one leg2 per (mc,d)
group on sync.  Transpose staging uses per-(mc,d) DRAM tensors.

  loss[b] = sum_t (ln Z + ln r) - sum_k ln z_f - sum_k ln z_b - ln P
            + 160 ln tau
"""

import functools
import os
import sys

import numpy as np

B, T, C, L = 512, 512, 128, 80
S = 2 * L + 1  # 161
BLANK = C - 1
EPS = 1e-7
NCORES = 8
BPC = B // NCORES  # 64 sequences per core
TC = 64  # time-chunk
NMC = 4  # chunks per direction (255 chain steps)
TSUB = 4  # times per partition-cell
NW = TC // TSUB  # 16 w-slots
NOCT = BPC // 8  # 8 seq-octets per core
NUNITS = NMC * 2 * NOCT  # 64 producer units (mc, dir, octet)
RN = 8
NRENORM = 31
SPAD = S + 2  # 163
SG = S + 3  # 164
SW = 81  # gather slot width: [blank D, 80 labels]
GW = TSUB * SW  # 324
IDX3W = 22  # idx cols (336/16=21, padded even for 32-bit idx words)
GWPAD = 336  # gather out width (num_valid must be 16-aligned)
RIDXW = 12
TAU = 0.3


def _emit_kernel(ctx, tc, ypred, idxt, maskt, mn2t, losst):
    import concourse.bass as bass  # noqa: F401
    import concourse.mybir as mybir
    from concourse.ap import AP

    nc = tc.nc
    f32 = mybir.dt.float32
    bf16 = mybir.dt.bfloat16
    Alu = mybir.AluOpType
    Act = mybir.ActivationFunctionType

    singles = ctx.enter_context(tc.tile_pool(name="singles", bufs=1))
    ypool = ctx.enter_context(tc.tile_pool(name="ypool", bufs=15))
    bpool = ctx.enter_context(tc.tile_pool(name="bpool", bufs=14))
    gpool = ctx.enter_context(tc.tile_pool(name="gpool", bufs=14))
    small = ctx.enter_context(tc.tile_pool(name="small", bufs=2))
    finp = ctx.enter_context(tc.tile_pool(name="finp", bufs=8))
    epi = ctx.enter_context(tc.tile_pool(name="epi", bufs=1))
    psump = ctx.enter_context(tc.tile_pool(name="psum", bufs=2, space="PSUM"))

    # --- constants loaded once -------------------------------------------
    NIDX = 2 * NOCT * IDX3W + RIDXW
    idx_sb = singles.tile([128, NIDX], mybir.dt.uint16)
    nc.sync.dma_start(out=idx_sb[:, :], in_=idxt)
    m_sb = singles.tile([128, S], bf16)
    nc.sync.dma_start(out=m_sb[:, :], in_=maskt)
    mn2_sb = singles.tile([128, SW - 1], bf16)
    nc.sync.dma_start(out=mn2_sb[:, :], in_=mn2t)
    gsel = singles.tile([128, 8], f32)
    nc.sync.dma_start(out=gsel[:, :], in_=tc.gselt)
    idx_scr = singles.tile([16, 1], mybir.dt.uint16)
    nc.gpsimd.tensor_copy(out=idx_scr[:, :], in_=idx_sb[0:16, 0:1])

    # Z accumulator: col = unit*TSUB + u, unit = (mc*2+dir)*NOCT + o
    zbig = singles.tile([128, NUNITS * TSUB], f32)
    zstash = singles.tile([128, NRENORM], f32)
    # 1/D per (row, step i): filled per chunk from the transposed blank slots
    rblock = singles.tile([128, NMC * TC], f32)

    gtiles = []
    for mc in range(NMC):
        gtile = singles.tile([128, TC * SW], bf16, tag=f"gtile{mc}")
        gtiles.append(gtile)

    # --- producer pipeline -----------------------------------------------
    # unit k = (mc, d, o).  HWDGE DMAs execute in FIFO order per issuing
    # engine ring, and a consumer waiting on DMA #n waits for every DMA
    # before it in that ring — so the three DMA streams are spread across
    # all three rings: octet-PAIR loads fwd on sync / bwd on act, per-unit
    # leg1 on gpsimd's SWDGE (issued right after its gather, never blocks),
    # and one leg2 per (mc,d) group on sync.
    def pair_load(kp):
        k = 2 * kp
        mc, r = divmod(k, 2 * NOCT)
        d, o = divmod(r, NOCT)
        yb = ypool.tile([128, 2 * TSUB * C], f32, tag="yb")
        if d == 0:
            off = 8 * o * T * C + (mc * TC) * C
            wstep = TSUB * C
        else:
            off = 8 * o * T * C + ((NMC * 2 - 1 - mc) * TC) * C + (NW - 1) * TSUB * C
            wstep = -TSUB * C
        eng = nc.sync if d == 0 else nc.scalar
        for j in range(2):
            src = AP(
                ypred.tensor,
                ypred.offset + off + j * 8 * T * C,
                [[T * C, 8], [wstep, NW], [1, TSUB * C]],
            )
            eng.dma_start(
                out=yb[:, j * TSUB * C : (j + 1) * TSUB * C], in_=src
            )
        return yb

    NPAIR = NUNITS // 2
    LOOKAHEAD = 13  # pairs
    ybigs = {}
    for kp in range(LOOKAHEAD):
        ybigs[kp] = pair_load(kp)
    for k in range(NUNITS):
        kp, j = divmod(k, 2)
        if j == 0 and kp + LOOKAHEAD < NPAIR:
            ybigs[kp + LOOKAHEAD] = pair_load(kp + LOOKAHEAD)
        yb = ybigs[kp] if j == 0 else ybigs.pop(kp)
        jo = j * TSUB * C
        mc, r = divmod(k, 2 * NOCT)
        d, o = divmod(r, NOCT)
        # label gather straight off the fp32 load (no cast in this loop);
        # the eps bias is applied by the chain-side emission math being
        # tolerant: we add eps in the Z accum below and in the gathered
        # values via the casting leg1?  No: eps rides the Z accum and the
        # chain's G1; gathered values are y (eps added numerically via the
        # +eps Z path cancels).  See module docstring.
        g2 = gpool.tile([128, GWPAD], f32, tag="g2")
        nc.gpsimd.tensor_copy(out=g2[0:16, 0:1], in_=yb[0:16, jo : jo + 1])
        ib = (d * NOCT + o) * IDX3W
        nc.gpsimd.indirect_copy(
            g2[:, :], yb[:, jo : jo + TSUB * C], idx_sb[:, ib : ib + IDX3W], True
        )
        # leg1 from gp's own SWDGE ring, right after the gather it reads;
        # the DMA itself casts fp32 -> bf16 (SWDGE-only feature)
        out_v = tc.gst[2 * mc + d][8 * o : 8 * o + 8, :].rearrange(
            "g (w q) -> g w q", w=NW
        )
        nc.gpsimd.dma_start(out=out_v, in_=g2[:, 0:GW])
        # Z accumulation: free-running act side branch, consumed by nobody
        # until the epilogue (dummy bf16 output tile)
        ydum = bpool.tile([128, TSUB * C], bf16, tag="yf")
        for u in range(TSUB):
            col = k * TSUB + u
            nc.scalar.activation(
                out=ydum[:, u * C : (u + 1) * C],
                in_=yb[:, jo + u * C : jo + (u + 1) * C],
                func=Act.Copy,
                bias=EPS,
                accum_out=zbig[:, col : col + 1],
            )
        # leg2 per (mc,d) group, 4 units after the group's last gather
        if k >= 11 and (k - 11) % 8 == 0:
            _leg2(nc, tc, gtiles, (k - 11) // 8)
    _leg2(nc, tc, gtiles, 2 * NMC - 1)

    # --- the DP chain (vector only) ---------------------------------------
    NLAB = SW - 1  # 80 label states
    Qo = singles.tile([128, NLAB], bf16)
    Qe = singles.tile([128, NLAB + 1], bf16)
    xo = singles.tile([128, NLAB], bf16)
    Rh = singles.tile([128, NLAB], bf16)

    def chunk_preamble(mc):
        # rblock[:, mc*TC : (mc+1)*TC] = 1 / (blank-slots of gtile[mc] + eps)
        blf = small.tile([128, TC], f32, tag="blf")
        nc.vector.tensor_scalar(
            out=blf[:, :], in0=gtiles[mc][:, 0 : TC * SW : SW], scalar1=EPS,
            scalar2=0.0, op0=Alu.add, op1=Alu.add,
        )
        nc.vector.reciprocal(
            out=rblock[:, mc * TC : (mc + 1) * TC], in_=blf[:, :]
        )

    chunk_preamble(0)
    nc.vector.memset(Qe[:, :], 0.0)
    nc.vector.memset(Qe[:, 0:1], 1.0)
    nc.vector.memset(Qo[:, :], 0.0)
    nc.vector.tensor_scalar(
        out=Qo[:, 0:1], in0=gtiles[0][:, 1:2], scalar1=rblock[:, 0:1],
        scalar2=TAU, op0=Alu.mult, op1=Alu.mult,
    )
    nc.vector.memset(Rh[:, :], 0.0)
    nc.vector.tensor_tensor(
        out=Rh[:, 0:1], in0=Qo[:, 0:1], in1=mn2_sb[:, 0:1], op=Alu.mult
    )

    rz = None
    for i in range(1, 256):
        mc, toff = divmod(i, TC)
        if toff == 0 and mc > 0:
            chunk_preamble(mc)
        g1 = gtiles[mc][:, toff * SW + 1 : (toff + 1) * SW]
        rt = rblock[:, i : i + 1]
        measure = (i % RN == RN - 1) and i < 255
        fold = (i % RN == 0) and i >= RN
        # (a) xo = Qo + tau*Qe
        nc.vector.scalar_tensor_tensor(
            out=xo[:, :], in0=Qe[:, 0:NLAB], scalar=TAU, in1=Qo[:, :],
            op0=Alu.mult, op1=Alu.add,
        )
        # (b) xo[1:] += Rh[:-1]   (skip transitions)
        nc.vector.tensor_tensor(
            out=xo[:, 1:NLAB], in0=xo[:, 1:NLAB], in1=Rh[:, 0 : NLAB - 1],
            op=Alu.add,
        )
        # (e) Qe[1:] += tau*Qo  (in place; Qe[0] is constant between folds)
        if measure:
            ze = small.tile([128, 1], f32, tag="ze")
            nc.vector.scalar_tensor_tensor(
                out=Qe[:, 1 : NLAB + 1], in0=Qo[:, :], scalar=TAU,
                in1=Qe[:, 1 : NLAB + 1], op0=Alu.mult, op1=Alu.add,
                accum_out=ze[:, :],
            )
        else:
            nc.vector.scalar_tensor_tensor(
                out=Qe[:, 1 : NLAB + 1], in0=Qo[:, :], scalar=TAU,
                in1=Qe[:, 1 : NLAB + 1], op0=Alu.mult, op1=Alu.add,
            )
        # (c) Qo = (xo * r_t) * G1  -- 1/D rides the scalar slot; renorm
        # folds multiply rz in as well
        if fold:
            rzr = small.tile([128, 1], f32, tag="rzr")
            nc.vector.tensor_tensor(
                out=rzr[:, :], in0=rz[:, :], in1=rt, op=Alu.mult
            )
            nc.vector.scalar_tensor_tensor(
                out=Qo[:, :], in0=xo[:, :], scalar=rzr[:, :], in1=g1,
                op0=Alu.mult, op1=Alu.mult,
            )
            # (f) fold even states too
            nc.vector.tensor_scalar(
                out=Qe[:, :], in0=Qe[:, :], scalar1=rz[:, :], scalar2=1.0,
                op0=Alu.mult, op1=Alu.mult,
            )
        elif measure:
            zo = small.tile([128, 1], f32, tag="zo")
            nc.vector.scalar_tensor_tensor(
                out=Qo[:, :], in0=xo[:, :], scalar=rt, in1=g1,
                op0=Alu.mult, op1=Alu.mult, accum_out=zo[:, :],
            )
        else:
            nc.vector.scalar_tensor_tensor(
                out=Qo[:, :], in0=xo[:, :], scalar=rt, in1=g1,
                op0=Alu.mult, op1=Alu.mult,
            )
        # (d) Rh = Qo * mn2
        nc.vector.tensor_tensor(
            out=Rh[:, :], in0=Qo[:, :], in1=mn2_sb[:, 0:NLAB], op=Alu.mult
        )
        if measure:
            kk = i // RN
            nc.vector.tensor_tensor(
                out=zstash[:, kk : kk + 1], in0=zo[:, :], in1=ze[:, :], op=Alu.add
            )
            rz = small.tile([128, 1], f32, tag="rz")
            nc.vector.reciprocal(out=rz[:, :], in_=zstash[:, kk : kk + 1])

    # --- epilogue ---------------------------------------------------------
    # rebuild interleaved full state (state s at col s+2, as v6)
    Qfull = singles.tile([128, SPAD], bf16)
    nc.vector.memset(Qfull[:, :], 0.0)
    nc.vector.tensor_copy(out=Qfull[:, 2:SPAD:2], in_=Qe[:, :])
    nc.vector.tensor_copy(out=Qfull[:, 3:SPAD:2], in_=Qo[:, :])
    # beta' step on bwd rows: bt = g~ + tau*g~[-1] + m~t*g~[-2]
    bx = epi.tile([64, S], bf16, tag="bx")
    nc.vector.scalar_tensor_tensor(
        out=bx[:, :], in0=Qfull[64:128, 1 : S + 1], scalar=TAU,
        in1=Qfull[64:128, 2:SPAD], op0=Alu.mult, op1=Alu.add,
    )
    by = epi.tile([64, S], bf16, tag="by")
    nc.vector.tensor_tensor(
        out=by[:, :], in0=m_sb[64:128, :], in1=Qfull[64:128, 0:S], op=Alu.mult
    )
    btfull = singles.tile([128, SG], bf16)
    nc.vector.memset(btfull[:, :], 0.0)
    nc.vector.tensor_tensor(
        out=btfull[64:128, 0:S], in0=bx[:, :], in1=by[:, :], op=Alu.add
    )
    # state-reversal gather
    btrev = singles.tile([128, SG], bf16)
    nc.gpsimd.tensor_copy(out=btrev[0:16, 0:1], in_=btfull[0:16, 0:1])
    jr = 2 * NOCT * IDX3W
    nc.gpsimd.indirect_copy(
        btrev[:, :], btfull[:, :], idx_sb[:, jr : jr + RIDXW], True
    )
    balign = epi.tile([64, S], bf16, tag="balign")
    nc.sync.dma_start(out=balign[:, :], in_=btrev[64:128, 0:S])
    # join dot: Phat[b] = sum_s alpha[b,s] * balign[b,s]
    pjunk = epi.tile([64, S], bf16, tag="pjunk")
    phat = finp.tile([64, 1], f32, tag="fin")
    nc.vector.scalar_tensor_tensor(
        out=pjunk[:, :], in0=Qfull[0:64, 2:SPAD], scalar=1.0, in1=balign[:, :],
        op0=Alu.mult, op1=Alu.mult, accum_out=phat[:, :],
    )
    lnp = finp.tile([64, 1], f32, tag="fin")
    nc.scalar.activation(out=lnp[:, :], in_=phat[:, :], func=Act.Ln)

    # renorm scale logs
    lnzt = epi.tile([128, NRENORM], f32, tag="lnzt")
    nc.scalar.activation(out=lnzt[:, :], in_=zstash[:, :], func=Act.Ln)
    rfull = epi.tile([128, 1], f32, tag="rfull")
    nc.vector.reduce_sum(out=rfull[:, :], in_=lnzt[:, :], axis=mybir.AxisListType.X)
    rb = finp.tile([64, 1], f32, tag="fin")
    nc.sync.dma_start(out=rb[:, :], in_=rfull[64:128, :])
    rsum = finp.tile([64, 1], f32, tag="fin")
    nc.vector.tensor_tensor(
        out=rsum[:, :], in0=rfull[0:64, :], in1=rb[:, :], op=Alu.add
    )

    # softmax normalizer W: ln(Z~), reduce (u) then (mcd), PE group-sum over w
    NMCD = 2 * NMC
    lnZ = singles.tile([128, NMCD * NOCT * TSUB], f32)
    nc.scalar.activation(out=lnZ[:, :], in_=zbig[:, :], func=Act.Ln)
    wt1 = singles.tile([128, NMCD * NOCT], f32)
    lv = lnZ[:, :].rearrange("p (m o u) -> p (m o) u", m=NMCD, o=NOCT)
    nc.vector.reduce_sum(out=wt1[:, :], in_=lv, axis=mybir.AxisListType.X)
    wsum3 = singles.tile([128, NOCT], f32)
    lv2 = wt1[:, :].rearrange("p (m o) -> p o m", m=NMCD)
    nc.vector.reduce_sum(out=wsum3[:, :], in_=lv2, axis=mybir.AxisListType.X)
    psw = psump.tile([8, 8], f32, tag="ps1")
    nc.tensor.matmul(psw[:, :], lhsT=gsel[:, :], rhs=wsum3[:, :], start=True, stop=True)
    wsb = epi.tile([8, 8], f32, tag="wsb")
    nc.vector.tensor_copy(out=wsb[:, :], in_=psw[:, :])
    wb = finp.tile([BPC, 1], f32, tag="fin")
    for o in range(NOCT):
        nc.sync.dma_start(out=wb[8 * o : 8 * o + 8, :], in_=wsb[:, o : o + 1])

    # W_r = sum_t ln(1/D_t) per chain row, fwd+bwd folded per seq
    lnrt = epi.tile([128, NMC * TC], f32, tag="lnrt")
    nc.scalar.activation(out=lnrt[:, :], in_=rblock[:, :], func=Act.Ln)
    wrfull = epi.tile([128, 1], f32, tag="wrfull")
    nc.vector.reduce_sum(out=wrfull[:, :], in_=lnrt[:, :], axis=mybir.AxisListType.X)
    wrb = finp.tile([64, 1], f32, tag="fin")
    nc.sync.dma_start(out=wrb[:, :], in_=wrfull[64:128, :])
    wradd = finp.tile([64, 1], f32, tag="fin")
    nc.vector.tensor_tensor(
        out=wradd[:, :], in0=wrfull[0:64, :], in1=wrb[:, :], op=Alu.add
    )

    # loss = W_Z + W_r - Rsum - lnP + 160 ln tau
    t0 = finp.tile([BPC, 1], f32, tag="fin")
    nc.vector.tensor_tensor(out=t0[:, :], in0=wb[:, :], in1=wradd[:, :], op=Alu.add)
    t1 = finp.tile([BPC, 1], f32, tag="fin")
    nc.vector.tensor_tensor(out=t1[:, :], in0=t0[:, :], in1=rsum[:, :], op=Alu.subtract)
    t2 = finp.tile([BPC, 1], f32, tag="fin")
    nc.vector.tensor_tensor(out=t2[:, :], in0=t1[:, :], in1=lnp[:, :], op=Alu.subtract)
    lt = finp.tile([BPC, 1], f32, tag="fin")
    nc.vector.tensor_scalar(
        out=lt[:, :], in0=t2[:, :], scalar1=float(S - 1) * float(np.log(TAU)),
        scalar2=0.0, op0=Alu.add, op1=Alu.add,
    )
    nc.sync.dma_start(out=losst, in_=lt[:, :])


def _leg2(nc, tc, gtiles, gidx):
    # gst[(mc,d)] [64 rows, TC*SW] -> gtile[mc] rows [64d : 64d+64]
    mc, d = divmod(gidx, 2)
    nc.sync.dma_start(
        out=gtiles[mc][64 * d : 64 * d + 64, :], in_=tc.gst[gidx][:, :]
    )


@functools.lru_cache(maxsize=4)
def _build():
    from contextlib import ExitStack

    import concourse.bacc as bacc
    import concourse.mybir as mybir
    import concourse.tile as tile

    nc = bacc.Bacc(trn_type="TRN2", target_bir_lowering=False)
    ypred = nc.dram_tensor("y_pred", [BPC, T, C], mybir.dt.float32, kind="ExternalInput")
    NIDX = 2 * NOCT * IDX3W + RIDXW
    idxt = nc.dram_tensor("idx", [128, NIDX], mybir.dt.uint16, kind="ExternalInput")
    maskt = nc.dram_tensor("mask", [128, S], mybir.dt.bfloat16, kind="ExternalInput")
    mn2t = nc.dram_tensor("mn2", [128, SW - 1], mybir.dt.bfloat16, kind="ExternalInput")
    gselt = nc.dram_tensor("gsel", [128, 8], mybir.dt.float32, kind="ExternalInput")
    losst = nc.dram_tensor("loss", [BPC, 1], mybir.dt.float32, kind="ExternalOutput")
    gst = [
        nc.dram_tensor(f"gst{g}", [64, TC * SW], mybir.dt.bfloat16, kind="Internal")
        for g in range(2 * NMC)
    ]
    with tile.TileContext(nc) as tc:
        tc.gselt = gselt[:, :]
        tc.gst = gst
        with ExitStack() as ctx:
            _emit_kernel(
                ctx, tc, ypred[:, :, :], idxt[:, :], maskt[:, :], mn2t[:, :],
                losst[:, :],
            )
    nc.compile()
    return nc


def _host_prep(y_true):
    """Dense 80-slot octet gather indices (fwd, bwd time+state reversed,
    epilogue state reversal), tau^2-scaled masks, and the dense skip mask."""
    import ml_dtypes

    bf = ml_dtypes.bfloat16
    lab = np.asarray(y_true).astype(np.int64)
    ext_m = np.zeros((B, S), dtype=np.float32)
    ext_m[:, 1] = 1.0
    ext_m[:, 3::2] = (lab[:, 1:] != lab[:, :-1]).astype(np.float32)
    mt = np.zeros((B, S), dtype=np.float32)  # m~[s'] = m[162-s']
    sp = np.arange(2, S)
    mt[:, sp] = ext_m[:, 162 - sp]
    t2 = np.float32(TAU * TAU)

    NIDX = 2 * NOCT * IDX3W + RIDXW
    p = np.arange(128)
    idx_all, mask_all, mn2_all = [], [], []
    for kcore in range(NCORES):
        base = kcore * BPC
        idx = np.zeros((128, NIDX), dtype=np.uint16)
        for d in range(2):
            for o in range(NOCT):
                ib = (d * NOCT + o) * IDX3W
                for f in range(IDX3W):
                    pos = f * 16 + (p % 16)  # position in the 324-list
                    valid = pos < GW
                    u = np.minimum(pos // SW, TSUB - 1)
                    slot = pos % SW
                    g = p // 16
                    seq = base + 8 * o + g
                    ueff = u if d == 0 else (TSUB - 1 - u)
                    val = np.zeros(128, dtype=np.uint16)
                    bl = valid & (slot == 0)
                    lb = valid & (slot >= 1)
                    val[bl] = (ueff[bl] * C + BLANK).astype(np.uint16)
                    if d == 0:
                        val[lb] = (
                            ueff[lb] * C + lab[seq[lb], slot[lb] - 1]
                        ).astype(np.uint16)
                    else:
                        val[lb] = (
                            ueff[lb] * C + lab[seq[lb], L - slot[lb]]
                        ).astype(np.uint16)
                    idx[:, ib + f] = val
        jr = 2 * NOCT * IDX3W
        for f in range(RIDXW):
            pos = f * 16 + (p % 16)
            valid = pos < S
            col = np.zeros(128, dtype=np.uint16)
            col[valid] = (S - 1) - pos[valid]
            idx[:, jr + f] = col
        idx_all.append(idx)

        mask = np.zeros((128, S), dtype=np.float32)
        mask[0:64] = ext_m[base : base + BPC] * t2
        mask[64:128] = mt[base : base + BPC] * t2
        mask_all.append(mask.astype(bf))
        # dense skip mask for Rh: mn2[r, j] = mask[r, 2j+3], 0 at j=79
        modd = mask[:, 1::2]
        mn2 = np.concatenate([modd[:, 1:80], np.zeros((128, 1), np.float32)], axis=1)
        mn2_all.append(np.ascontiguousarray(mn2).astype(bf))
    return idx_all, mask_all, mn2_all


def gsel_host():
    g = np.zeros((128, 8), dtype=np.float32)
    for gg in range(8):
        g[16 * gg : 16 * gg + 16, gg] = 1.0
    return g


def kernel(y_true, y_pred):
    from concourse.bass_utils import run_bass_kernel_spmd

    y_pred = np.ascontiguousarray(np.asarray(y_pred), dtype=np.float32)
    idx_all, mask_all, mn2_all = _host_prep(y_true)

    nc = _build()
    in_maps = []
    for k in range(NCORES):
        b0 = k * BPC
        in_maps.append(
            {
                "y_pred": np.ascontiguousarray(y_pred[b0 : b0 + BPC]),
                "idx": idx_all[k],
                "mask": mask_all[k],
                "mn2": mn2_all[k],
                "gsel": gsel_host(),
            }
        )
    res = run_bass_kernel_spmd(
        nc,
        in_maps,
        core_ids=list(range(NCORES)),
        trace=bool(int(os.environ.get("CTC_TRACE", "0"))),
    )
    out = np.concatenate([r["loss"] for r in res.results], axis=0)
    if res.exec_time_ns is not None:
        print(f"HW exec time: {res.exec_time_ns} ns", file=sys.stderr)
    return out.astype(np.float32)
